# revision 6
# baseline (speedup 1.0000x reference)
# Trainium2 Bass kernel for nn_DepthCorr (SiamRPN-style depthwise correlation head).
#
# Pipeline (per batch):
#   kf   = relu(bn(conv3x3(kernel, Wk)))   [C=256, 7,7]  -> [H=256, 5,5]
#   sf   = relu(bn(conv3x3(search, Ws)))   [C=256,31,31] -> [H=256,29,29]
#   corr = relu(dwxcorr(sf, kf))                         -> [H=256,25,25]
#   out  = relu(bn(conv1x1(corr, Wf)))                   -> [C=256,25,25]
#
# Sharding: pure data-parallel over batch (128 batches / 8 cores = 16 per core).
# BN is folded into conv weights + per-channel bias on the host.
#
# Engine split (the key idea vs the diag-matmul baseline): the dense convs
# (conv1/conv2/conv3) run on the tensor engine, while the depthwise xcorr --
# which can only use 128 of the PE's 16K MACs per cycle via diagonal weights --
# runs entirely on the Vector (DVE) and GpSimd engines as per-tap fused
# multiply-accumulates: scalar_tensor_tensor(acc, sf_window, kf[c,t], acc).
# The 25 taps are split between DVE (fast, 2x fp16 mode) and GpSimd; each
# accumulates its taps into its own fp16 accumulator, then DVE merges and
# the scalar engine applies relu.  This overlaps the whole xcorr with the
# next batch's conv2 on the PE, roughly halving the critical path.
#
# DVE 2x mode needs 4B-aligned step-1 operands, so odd-tj tap windows read
# from a one-column-shifted copy of sf (sfs) made by the scalar engine.

import numpy as np
from contextlib import ExitStack

import concourse.bass as bass
import concourse.mybir as mybir
import concourse.tile as tile
from concourse import bacc
from concourse.bass_utils import run_bass_kernel_spmd

B, C, H = 128, 256, 256
N_CORES = 8
NB = B // N_CORES  # batches per core
EPS = 1e-5
FP = mybir.dt.float32
FR = mybir.dt.float32r
RELU = mybir.ActivationFunctionType.Relu
COPY = mybir.ActivationFunctionType.Copy
F16 = mybir.dt.float16
MUL = mybir.AluOpType.mult
ADD = mybir.AluOpType.add

# xcorr tap split: GpSimd takes 2 odd-tj taps (alignment-agnostic, but slow --
# plain tensor_tensor at ~0.42 efficiency), DVE takes the rest with the fused
# scalar_tensor_tensor; odd-tj DVE taps read the shifted sf copy for 4B align.
ALL_TAPS = [(ti, tj) for ti in range(5) for tj in range(5)]
POOL_TAPS = [(1, 1), (3, 3)]
DVE_TAPS = [t for t in ALL_TAPS if t not in POOL_TAPS]


def _build_nc(nb=NB):
    nc = bacc.Bacc()

    search = nc.declare_dram_parameter("search", [nb, C, 31, 32], F16, isOutput=False)
    # kin pre-transposed on the host to [k, cg, h, w, b] so the DMA is contiguous
    kin = nc.declare_dram_parameter("kin", [128, 2, 7, 7, nb], FR, isOutput=False)
    wk_d = nc.declare_dram_parameter("wk", [128, 36, 128], FR, isOutput=False)
    ws_d = nc.declare_dram_parameter("ws", [128, 36, 128], F16, isOutput=False)
    wf_d = nc.declare_dram_parameter("wf", [128, 4, 128], F16, isOutput=False)
    bias_d = nc.declare_dram_parameter("bias", [128, 6], FP, isOutput=False)
    out_d = nc.declare_dram_parameter("out", [nb, C, 25, 25], FP, isOutput=True)

    C2_SPLITS = [(0, 16), (16, 13)]  # conv2 row splits; N = 464 / 377
    O_SPLITS = [(0, 13), (13, 12)]  # conv3 row splits over corr; N = 325 / 300

    with tile.TileContext(nc) as tc, ExitStack() as ctx:
        wpool = ctx.enter_context(tc.tile_pool(name="wpool", bufs=1))
        kpool = ctx.enter_context(tc.tile_pool(name="kpool", bufs=1))
        spool = ctx.enter_context(tc.tile_pool(name="spool", bufs=3))
        fpool = ctx.enter_context(tc.tile_pool(name="fpool", bufs=2))
        gpool = ctx.enter_context(tc.tile_pool(name="gpool", bufs=2))
        apool = ctx.enter_context(tc.tile_pool(name="apool", bufs=2))
        bpool = ctx.enter_context(tc.tile_pool(name="bpool", bufs=2))
        cpool = ctx.enter_context(tc.tile_pool(name="cpool", bufs=2))
        p2pool = ctx.enter_context(tc.tile_pool(name="p2pool", bufs=2))
        opool = ctx.enter_context(tc.tile_pool(name="opool", bufs=2))
        ps_c = ctx.enter_context(tc.tile_pool(name="ps_c", bufs=4, space="PSUM"))
        ps_o = ctx.enter_context(tc.tile_pool(name="ps_o", bufs=4, space="PSUM"))

        # --- search prefetch (sync queue; weights go on gpsimd queue) ---
        s_tiles = {}

        def load_search(b):
            s_sb = spool.tile([128, 2, 31, 32], F16, tag="sin")
            nc.sync.dma_start(out=s_sb[:, 0, :, :], in_=search[b, 0:128, :, :])
            nc.scalar.dma_start(out=s_sb[:, 1, :, :], in_=search[b, 128:256, :, :])
            s_tiles[b] = s_sb

        # --- resident constants; conv2 weights + bias race ahead of the
        # search prefetch so the PE can start within ~10us ---
        wk_sb = wpool.tile([128, 36, 128], FR, tag="wk")
        ws_sb = wpool.tile([128, 36, 128], F16, tag="ws")
        wf_sb = wpool.tile([128, 4, 128], F16, tag="wf")
        bias_sb = wpool.tile([128, 6], FP, tag="bias")
        load_search(0)
        nc.gpsimd.dma_start(out=ws_sb[:], in_=ws_d[:])
        nc.scalar.dma_start(out=bias_sb[:], in_=bias_d[:])
        k_sbs = []
        for cg in range(2):
            k_sb = kpool.tile([128, 7, 7, nb], FR, tag=f"kin{cg}")
            k_sbs.append(k_sb)
        # kf_sb[h_part, hg, tap, b]
        kf_sb = kpool.tile([128, 2, 25, nb], FP, tag="kf")

        def load_deferred_consts():
            nc.gpsimd.dma_start(out=wk_sb[:], in_=wk_d[:])
            nc.gpsimd.dma_start(out=wf_sb[:], in_=wf_d[:])
            for cg in range(2):
                nc.gpsimd.dma_start(out=k_sbs[cg][:], in_=kin[:, cg])

        def conv1():
            for hg in range(2):
                ps = ps_c.tile([128, 5, 5, nb], FP, tag="psc")
                n_mm = 0
                for cg in range(2):
                    for dy in range(3):
                        for dx in range(3):
                            t = dy * 3 + dx
                            nc.tensor.matmul(
                                ps[:],
                                lhsT=wk_sb[:, hg * 18 + t * 2 + cg, :],
                                rhs=k_sbs[cg][:, dy:dy + 5, dx:dx + 5, :],
                                start=(n_mm == 0),
                                stop=(n_mm == 17),
                            )
                            n_mm += 1
                nc.scalar.activation(
                    out=kf_sb[:, hg, :, :],
                    in_=ps.rearrange("p a b c -> p (a b) c"),
                    func=RELU,
                    bias=bias_sb[:, 0 + hg:1 + hg],
                    scale=1.0,
                )

        def conv3_and_store(b, corr):
            out_sb = opool.tile([128, 2, 25, 25], FP, tag="osb")
            for og in range(2):
                for (r0, nr) in O_SPLITS:
                    ps = ps_o.tile([128, nr, 25], FP, tag="pso")
                    for hg in range(2):
                        nc.tensor.matmul(
                            ps[:],
                            lhsT=wf_sb[:, hg * 2 + og, :],
                            rhs=corr[:, hg, r0:r0 + nr, 0:25],
                            start=(hg == 0),
                            stop=(hg == 1),
                        )
                    nc.scalar.activation(
                        out=out_sb[:, og, r0:r0 + nr, :],
                        in_=ps[:],
                        func=RELU,
                        bias=bias_sb[:, 4 + og:5 + og],
                        scale=1.0,
                    )
                nc.sync.dma_start(
                    out=out_d[b, og * 128:(og + 1) * 128, :, :],
                    in_=out_sb[:, og, :, :],
                )

        # --- per-batch main pipeline.  conv3 for batch b-1 is emitted after
        # batch b's conv2+xcorr so the PE never waits on the xcorr engines. ---
        corr_prev = None
        for b in range(nb):
            if b + 1 < nb:
                load_search(b + 1)
            s_sb = s_tiles.pop(b)

            if b == 0:
                load_deferred_consts()

            # conv2: search branch -> sf [h_part, hg, 29, 30] (col 29 unused pad)
            sf = fpool.tile([128, 2, 29, 30], F16, tag="sf")
            for hg in range(2):
                for (y0, ny) in C2_SPLITS:
                    ps = ps_c.tile([128, ny, 29], FP, tag="psc")
                    n_mm = 0
                    for cg in range(2):
                        for dy in range(3):
                            for dx in range(3):
                                t = dy * 3 + dx
                                nc.tensor.matmul(
                                    ps[:],
                                    lhsT=ws_sb[:, hg * 18 + t * 2 + cg, :],
                                    rhs=s_sb[
                                        :, cg, dy + y0:dy + y0 + ny, dx:dx + 29
                                    ],
                                    start=(n_mm == 0),
                                    stop=(n_mm == 17),
                                )
                                n_mm += 1
                    nc.scalar.activation(
                        out=sf[:, hg, y0:y0 + ny, 0:29],
                        in_=ps[:],
                        func=RELU,
                        bias=bias_sb[:, 2 + hg:3 + hg],
                        scale=1.0,
                    )
            if b == 0:
                conv1()

            # sf shifted left one column (4B-aligned windows for odd-tj taps)
            sfs = gpool.tile([128, 2, 29, 30], F16, tag="sfs")
            nc.scalar.activation(
                out=sfs[:, :, :, 0:28], in_=sf[:, :, :, 1:29], func=COPY,
            )

            # depthwise xcorr on DVE + GpSimd: acc/acc2[c, hg, 25, 26(pad)]
            acc = apool.tile([128, 2, 25, 26], F16, tag="acc")
            acc2 = bpool.tile([128, 2, 25, 26], F16, tag="acc2")
            corr = cpool.tile([128, 2, 25, 26], F16, tag="corr")
            for hg in range(2):
                for i, (ti, tj) in enumerate(DVE_TAPS):
                    if tj % 2 == 0:
                        in0 = sf[:, hg, ti:ti + 25, tj:tj + 25]
                    else:
                        in0 = sfs[:, hg, ti:ti + 25, tj - 1:tj + 24]
                    sc = kf_sb[:, hg, ti * 5 + tj, b:b + 1]
                    if i == 0:
                        nc.vector.tensor_scalar_mul(acc[:, hg, :, 0:25], in0, sc)
                    else:
                        nc.vector.scalar_tensor_tensor(
                            acc[:, hg, :, 0:25], in0, sc, acc[:, hg, :, 0:25],
                            op0=MUL, op1=ADD,
                        )
                for i, (ti, tj) in enumerate(POOL_TAPS):
                    in0 = sf[:, hg, ti:ti + 25, tj:tj + 25]
                    kb = (
                        kf_sb[:, hg, ti * 5 + tj, b:b + 1]
                        .unsqueeze(1).broadcast_to([128, 25, 25])
                    )
                    if i == 0:
                        nc.gpsimd.tensor_mul(acc2[:, hg, :, 0:25], in0, kb)
                    else:
                        tmp2 = p2pool.tile([128, 25, 25], F16, tag="tmp2")
                        nc.gpsimd.tensor_mul(tmp2[:], in0, kb)
                        nc.gpsimd.tensor_add(
                            acc2[:, hg, :, 0:25], acc2[:, hg, :, 0:25], tmp2[:]
                        )
            # merge partials (DVE), then relu (scalar engine)
            nc.vector.tensor_add(
                corr[:, :, :, 0:25], acc[:, :, :, 0:25], acc2[:, :, :, 0:25]
            )
            nc.scalar.activation(
                out=corr[:, :, :, 0:25], in_=corr[:, :, :, 0:25], func=RELU,
            )

            if corr_prev is not None:
                conv3_and_store(b - 1, corr_prev)
            corr_prev = corr
        conv3_and_store(nb - 1, corr_prev)

    nc.compile()
    return nc


def _fold_bn(W, g, be, m, v):
    inv = (g.astype(np.float64) / np.sqrt(v.astype(np.float64) + EPS))
    Wp = (W.astype(np.float64) * inv[:, None, None, None]).astype(np.float32)
    bp = (be.astype(np.float64) - m.astype(np.float64) * inv).astype(np.float32)
    return Wp, bp


def _pack_weights(Wk, gk, bk, mk, vk, Ws, gs, bs, ms, vs, Wf, gf, bf, mf, vf):
    Wkp, bkp = _fold_bn(Wk, gk, bk, mk, vk)
    Wsp, bsp = _fold_bn(Ws, gs, bs, ms, vs)
    Wfp, bfp = _fold_bn(Wf, gf, bf, mf, vf)

    def pack33(Wp):  # [H, C, 3, 3] -> [k, (hg, t, cg), m]
        w = Wp.reshape(2, 128, 2, 128, 3, 3)  # hg, m, cg, k, dy, dx
        w = w.transpose(3, 0, 4, 5, 2, 1)  # k, hg, dy, dx, cg, m
        return np.ascontiguousarray(w.reshape(128, 36, 128))

    wk_h = pack33(Wkp)
    ws_h = pack33(Wsp).astype(np.float16)
    w = Wfp[:, :, 0, 0].reshape(2, 128, 2, 128)  # og, m, hg, k
    wf_h = np.ascontiguousarray(
        w.transpose(3, 2, 0, 1).reshape(128, 4, 128)).astype(np.float16)

    bias_h = np.zeros((128, 6), np.float32)
    bias_h[:, 0] = bkp[0:128]
    bias_h[:, 1] = bkp[128:256]
    bias_h[:, 2] = bsp[0:128]
    bias_h[:, 3] = bsp[128:256]
    bias_h[:, 4] = bfp[0:128]
    bias_h[:, 5] = bfp[128:256]
    return wk_h, ws_h, wf_h, bias_h


_NC_CACHE = {}


def _get_nc(nb):
    if nb not in _NC_CACHE:
        _NC_CACHE[nb] = _build_nc(nb)
    return _NC_CACHE[nb]


def run(inputs, trace=False):
    """Build in_maps, run on 8 cores, return (full_output, BassKernelResults)."""
    kernel = np.asarray(inputs["kernel"], np.float32)
    search = np.asarray(inputs["search"], np.float32)
    wk_h, ws_h, wf_h, bias_h = _pack_weights(
        np.asarray(inputs["Wk"]), np.asarray(inputs["gk"]), np.asarray(inputs["bk"]),
        np.asarray(inputs["mk"]), np.asarray(inputs["vk"]),
        np.asarray(inputs["Ws"]), np.asarray(inputs["gs"]), np.asarray(inputs["bs"]),
        np.asarray(inputs["ms"]), np.asarray(inputs["vs"]),
        np.asarray(inputs["Wf"]), np.asarray(inputs["gf"]), np.asarray(inputs["bf"]),
        np.asarray(inputs["mf"]), np.asarray(inputs["vf"]),
    )
    nc = _get_nc(NB)
    # fp16 on host: identical to the on-device cast the kernel used to do
    search_p = np.zeros((B, C, 31, 32), np.float16)
    search_p[:, :, :, :31] = search
    in_maps = []
    for i in range(N_CORES):
        kk = kernel[i * NB:(i + 1) * NB].reshape(NB, 2, 128, 7, 7)
        kin_h = np.ascontiguousarray(kk.transpose(2, 1, 3, 4, 0))
        in_maps.append({
            "search": np.ascontiguousarray(search_p[i * NB:(i + 1) * NB]),
            "kin": kin_h,
            "wk": wk_h, "ws": ws_h, "wf": wf_h, "bias": bias_h,
        })
    res = run_bass_kernel_spmd(
        nc, in_maps, core_ids=list(range(N_CORES)), trace=trace
    )
    out = np.concatenate([res.results[i]["out"] for i in range(N_CORES)], axis=0)
    return out, res


def kernel(**inputs):
    out, _ = run(inputs, trace=False)
    return out


# revision 16
# speedup vs baseline: 1.4855x; 1.4855x over previous
# Trainium2 Bass kernel for nn_DepthCorr (SiamRPN-style depthwise correlation head).
#
# Pipeline (per batch):
#   kf   = relu(bn(conv3x3(kernel, Wk)))   [C=256, 7,7]  -> [H=256, 5,5]
#   sf   = relu(bn(conv3x3(search, Ws)))   [C=256,31,31] -> [H=256,29,29]
#   corr = relu(dwxcorr(sf, kf))                         -> [H=256,25,25]
#   out  = relu(bn(conv1x1(corr, Wf)))                   -> [C=256,25,25]
#
# Sharding: pure data-parallel over batch (128 batches / 8 cores = 16 per core).
# BN is folded into conv weights + per-channel bias on the host.
#
# Engine split (the key idea vs the diag-matmul baseline): the dense convs
# (conv1/conv2/conv3) run on the tensor engine, while the depthwise xcorr --
# which can only use 128 of the PE's 16K MACs per cycle via diagonal weights --
# runs entirely on the Vector (DVE) and GpSimd engines as per-tap fused
# multiply-accumulates: scalar_tensor_tensor(acc, sf_window, kf[c,t], acc).
# The 25 taps are split between DVE (fast, 2x fp16 mode) and GpSimd; each
# accumulates its taps into its own fp16 accumulator, then DVE merges and
# the scalar engine applies relu.  This overlaps the whole xcorr with the
# next batch's conv2 on the PE, roughly halving the critical path.
#
# DVE 2x mode needs 4B-aligned step-1 operands, so odd-tj tap windows read
# from a one-column-shifted copy of sf (sfs) made by the scalar engine.

import numpy as np
from contextlib import ExitStack

import concourse.bass as bass
import concourse.mybir as mybir
import concourse.tile as tile
from concourse import bacc
from concourse.bass_utils import run_bass_kernel_spmd

B, C, H = 128, 256, 256
N_CORES = 8
NB = B // N_CORES  # batches per core
EPS = 1e-5
FP = mybir.dt.float32
FR = mybir.dt.float32r
RELU = mybir.ActivationFunctionType.Relu
COPY = mybir.ActivationFunctionType.Copy
F16 = mybir.dt.float16
MUL = mybir.AluOpType.mult
ADD = mybir.AluOpType.add

# xcorr tap split across engines (per-tap costs measured on HW, both hg):
#   PE    diag-matmul                    ~0.53us -> 8 taps (t=5..12 contiguous)
#   DVE   TS-mult (4x) + TT-add (2x)     ~1.60us -> 8 even-tj taps
#   ACT   mult (scale=kf) + DVE TT-add   ~1.79+0.84us -> 7 taps
#   Pool  TT mult+add (bcast kf)         ~7.1us  -> 2 taps
# The offloaded partials (acc fed by DVE adds, acc2 by Pool) are folded back
# into the PE's xcorr PSUM group with identity matmuls, so the existing relu
# activation epilogue merges everything for free.  The diag build (one big
# broadcast multiply) runs on the otherwise-idle Pool engine.
PE_TAPS = [(1, 0), (1, 1), (1, 2), (1, 3), (1, 4), (2, 0), (2, 1), (2, 2)]
PE_T0 = 5  # first PE tap index (contiguous run t=5..12 in kf's tap dim)
POOL_TAPS = [(3, 1), (3, 3)]
TS_TAPS = [(0, 0), (0, 2), (0, 4), (3, 0), (3, 2), (3, 4), (4, 0), (4, 2)]
AV_TAPS = [(0, 1), (0, 3), (2, 3), (2, 4), (4, 1), (4, 3), (4, 4)]


def _build_nc(nb=NB):
    nc = bacc.Bacc()

    search = nc.declare_dram_parameter("search", [nb, C, 31, 32], F16, isOutput=False)
    # kin pre-transposed on the host to [k, cg, h, w, b] so the DMA is contiguous
    kin = nc.declare_dram_parameter("kin", [128, 2, 7, 7, nb], FR, isOutput=False)
    wk_d = nc.declare_dram_parameter("wk", [128, 36, 128], FR, isOutput=False)
    ws_d = nc.declare_dram_parameter("ws", [128, 36, 128], F16, isOutput=False)
    wf_d = nc.declare_dram_parameter("wf", [128, 4, 128], F16, isOutput=False)
    bias_d = nc.declare_dram_parameter("bias", [128, 6], FP, isOutput=False)
    eye_d = nc.declare_dram_parameter("eye", [128, 128], F16, isOutput=False)
    out_d = nc.declare_dram_parameter("out", [nb, C, 25, 25], FP, isOutput=True)

    C2_SPLITS = [(0, 16), (16, 13)]  # conv2 row splits; N = 464 / 377
    O_SPLITS = [(0, 13), (13, 12)]  # conv3 row splits over corr; N = 325 / 300

    with tile.TileContext(nc) as tc, ExitStack() as ctx:
        wpool = ctx.enter_context(tc.tile_pool(name="wpool", bufs=1))
        kpool = ctx.enter_context(tc.tile_pool(name="kpool", bufs=1))
        spool = ctx.enter_context(tc.tile_pool(name="spool", bufs=3))
        fpool = ctx.enter_context(tc.tile_pool(name="fpool", bufs=3))
        dpool = ctx.enter_context(tc.tile_pool(name="dpool", bufs=3))
        apool = ctx.enter_context(tc.tile_pool(name="apool", bufs=3))
        bpool = ctx.enter_context(tc.tile_pool(name="bpool", bufs=3))
        cpool = ctx.enter_context(tc.tile_pool(name="cpool", bufs=2))
        tpool = ctx.enter_context(tc.tile_pool(name="tpool", bufs=4))
        p2pool = ctx.enter_context(tc.tile_pool(name="p2pool", bufs=2))
        opool = ctx.enter_context(tc.tile_pool(name="opool", bufs=2))
        ps_c = ctx.enter_context(tc.tile_pool(name="ps_c", bufs=2, space="PSUM"))
        ps_x = ctx.enter_context(tc.tile_pool(name="ps_x", bufs=2, space="PSUM"))
        ps_o = ctx.enter_context(tc.tile_pool(name="ps_o", bufs=2, space="PSUM"))

        # --- search prefetch (sync queue; weights go on gpsimd queue) ---
        s_tiles = {}

        def load_search(b):
            s_sb = spool.tile([128, 2, 31, 32], F16, tag="sin")
            nc.sync.dma_start(out=s_sb[:, 0, :, :], in_=search[b, 0:128, :, :])
            nc.scalar.dma_start(out=s_sb[:, 1, :, :], in_=search[b, 128:256, :, :])
            s_tiles[b] = s_sb

        # --- resident constants; conv2 weights + bias race ahead of the
        # search prefetch so the PE can start within ~10us ---
        wk_sb = wpool.tile([128, 36, 128], FR, tag="wk")
        ws_sb = wpool.tile([128, 36, 128], F16, tag="ws")
        wf_sb = wpool.tile([128, 4, 128], F16, tag="wf")
        bias_sb = wpool.tile([128, 6], FP, tag="bias")
        eye_sb = wpool.tile([128, 128], F16, tag="eye")
        load_search(0)
        nc.gpsimd.dma_start(out=ws_sb[:], in_=ws_d[:])
        nc.scalar.dma_start(out=bias_sb[:], in_=bias_d[:])
        nc.scalar.dma_start(out=eye_sb[:], in_=eye_d[:])
        k_sbs = []
        for cg in range(2):
            k_sb = kpool.tile([128, 7, 7, nb], FR, tag=f"kin{cg}")
            k_sbs.append(k_sb)
        # kf_sb[h_part, hg, tap, b]
        kf_sb = kpool.tile([128, 2, 25, nb], FP, tag="kf")

        def load_deferred_consts():
            nc.gpsimd.dma_start(out=wk_sb[:], in_=wk_d[:])
            nc.gpsimd.dma_start(out=wf_sb[:], in_=wf_d[:])
            for cg in range(2):
                nc.gpsimd.dma_start(out=k_sbs[cg][:], in_=kin[:, cg])

        def conv1():
            for hg in range(2):
                ps = ps_c.tile([128, 5, 5, nb], FP, tag="psc")
                n_mm = 0
                for cg in range(2):
                    for dy in range(3):
                        for dx in range(3):
                            t = dy * 3 + dx
                            nc.tensor.matmul(
                                ps[:],
                                lhsT=wk_sb[:, hg * 18 + t * 2 + cg, :],
                                rhs=k_sbs[cg][:, dy:dy + 5, dx:dx + 5, :],
                                start=(n_mm == 0),
                                stop=(n_mm == 17),
                            )
                            n_mm += 1
                nc.scalar.activation(
                    out=kf_sb[:, hg, :, :],
                    in_=ps.rearrange("p a b c -> p (a b) c"),
                    func=RELU,
                    bias=bias_sb[:, 0 + hg:1 + hg],
                    scale=1.0,
                )

        def conv3_and_store(b, corr):
            out_sb = opool.tile([128, 2, 25, 25], FP, tag="osb")
            for og in range(2):
                for (r0, nr) in O_SPLITS:
                    ps = ps_o.tile([128, nr, 25], FP, tag="pso")
                    for hg in range(2):
                        nc.tensor.matmul(
                            ps[:],
                            lhsT=wf_sb[:, hg * 2 + og, :],
                            rhs=corr[:, hg, r0:r0 + nr, 0:25],
                            start=(hg == 0),
                            stop=(hg == 1),
                        )
                    nc.scalar.activation(
                        out=out_sb[:, og, r0:r0 + nr, :],
                        in_=ps[:],
                        func=RELU,
                        bias=bias_sb[:, 4 + og:5 + og],
                        scale=1.0,
                    )
                nc.sync.dma_start(
                    out=out_d[b, og * 128:(og + 1) * 128, :, :],
                    in_=out_sb[:, og, :, :],
                )

        def xcorr_pe(b, sf, diag, acc, acc2):
            # PE taps accumulate in PSUM, then the offloaded partials are
            # added with identity matmuls; relu epilogue merges to corr.
            corr = cpool.tile([128, 2, 25, 28], F16, tag="corr")
            for hg in range(2):
                for (r0, nr) in O_SPLITS:
                    ps = ps_x.tile([128, nr, 25], FP, tag="psx")
                    for i, (ti, tj) in enumerate(PE_TAPS):
                        nc.tensor.matmul(
                            ps[:],
                            lhsT=diag[:, hg, i, :],
                            rhs=sf[:, hg, ti + r0:ti + r0 + nr, tj:tj + 25],
                            start=(i == 0),
                            stop=False,
                        )
                    nc.tensor.matmul(
                        ps[:], lhsT=eye_sb[:],
                        rhs=acc[:, hg, r0:r0 + nr, 0:25],
                        start=False, stop=False,
                    )
                    nc.tensor.matmul(
                        ps[:], lhsT=eye_sb[:],
                        rhs=acc2[:, hg, r0:r0 + nr, 0:25],
                        start=False, stop=True,
                    )
                    nc.scalar.activation(
                        out=corr[:, hg, r0:r0 + nr, 0:25],
                        in_=ps[:],
                        func=RELU,
                        scale=1.0,
                    )
            return corr

        # --- per-batch main pipeline, software-pipelined with lag 2: the PE
        # xcorr + conv3 for batch b-2 are emitted after batch b's conv2 and
        # offloaded taps, so the PE never waits on the slower xcorr engines. ---
        state = {}
        for b in range(nb):
            if b + 1 < nb:
                load_search(b + 1)
            s_sb = s_tiles.pop(b)

            if b == 0:
                load_deferred_consts()

            # conv2: search branch -> sf [h_part, hg, 29, 30] (col 29 unused pad)
            sf = fpool.tile([128, 2, 29, 30], F16, tag="sf")
            for hg in range(2):
                for (y0, ny) in C2_SPLITS:
                    ps = ps_c.tile([128, ny, 29], FP, tag="psc")
                    n_mm = 0
                    for cg in range(2):
                        for dy in range(3):
                            for dx in range(3):
                                t = dy * 3 + dx
                                nc.tensor.matmul(
                                    ps[:],
                                    lhsT=ws_sb[:, hg * 18 + t * 2 + cg, :],
                                    rhs=s_sb[
                                        :, cg, dy + y0:dy + y0 + ny, dx:dx + 29
                                    ],
                                    start=(n_mm == 0),
                                    stop=(n_mm == 17),
                                )
                                n_mm += 1
                    nc.scalar.activation(
                        out=sf[:, hg, y0:y0 + ny, 0:29],
                        in_=ps[:],
                        func=RELU,
                        bias=bias_sb[:, 2 + hg:3 + hg],
                        scale=1.0,
                    )
            if b == 0:
                conv1()

            # diag for the PE taps: diag[c, hg, i, m] = kf[c, t5+i] * (c == m)
            # (built on the Pool engine; 1x broadcast multiply)
            diag = dpool.tile([128, 2, len(PE_TAPS), 128], F16, tag="diag")
            nc.gpsimd.tensor_mul(
                diag[:],
                kf_sb[:, :, PE_T0:PE_T0 + len(PE_TAPS), b]
                .unsqueeze(3).broadcast_to([128, 2, len(PE_TAPS), 128]),
                eye_sb.unsqueeze(1).unsqueeze(1)
                .broadcast_to([128, 2, len(PE_TAPS), 128]),
            )

            # offloaded xcorr taps -> acc (DVE adds) and acc2 (Pool)
            acc = apool.tile([128, 2, 25, 28], F16, tag="acc")
            acc2 = bpool.tile([128, 2, 25, 28], F16, tag="acc2")
            for i, (ti, tj) in enumerate(AV_TAPS):
                t = ti * 5 + tj
                dst = acc if i == 0 else tpool.tile([128, 2, 25, 28], F16, tag="tmp")
                for hg in range(2):
                    nc.scalar.activation(
                        out=dst[:, hg, :, 0:25],
                        in_=sf[:, hg, ti:ti + 25, tj:tj + 25],
                        func=COPY,
                        scale=kf_sb[:, hg, t, b:b + 1],
                    )
                if i > 0:
                    nc.vector.tensor_add(
                        acc[:, :, :, 0:25], acc[:, :, :, 0:25], dst[:, :, :, 0:25]
                    )
            for (ti, tj) in TS_TAPS:
                t = ti * 5 + tj
                tmp = tpool.tile([128, 2, 25, 28], F16, tag="tmp")
                for hg in range(2):
                    nc.vector.tensor_scalar_mul(
                        tmp[:, hg, :, 0:25],
                        sf[:, hg, ti:ti + 25, tj:tj + 25],
                        kf_sb[:, hg, t, b:b + 1],
                    )
                nc.vector.tensor_add(
                    acc[:, :, :, 0:25], acc[:, :, :, 0:25], tmp[:, :, :, 0:25]
                )
            for i, (ti, tj) in enumerate(POOL_TAPS):
                t = ti * 5 + tj
                for hg in range(2):
                    kb = (
                        kf_sb[:, hg, t, b:b + 1]
                        .unsqueeze(1).broadcast_to([128, 25, 25])
                    )
                    in0 = sf[:, hg, ti:ti + 25, tj:tj + 25]
                    if i == 0:
                        nc.gpsimd.tensor_mul(acc2[:, hg, :, 0:25], in0, kb)
                    else:
                        tmp2 = p2pool.tile([128, 25, 25], F16, tag="tmp2")
                        nc.gpsimd.tensor_mul(tmp2[:], in0, kb)
                        nc.gpsimd.tensor_add(
                            acc2[:, hg, :, 0:25], acc2[:, hg, :, 0:25], tmp2[:]
                        )
            state[b] = (sf, diag, acc, acc2)

            if b >= 2:
                corr = xcorr_pe(b - 2, *state.pop(b - 2))
                conv3_and_store(b - 2, corr)
        for bb in (nb - 2, nb - 1):
            corr = xcorr_pe(bb, *state.pop(bb))
            conv3_and_store(bb, corr)

    nc.compile()
    return nc


def _fold_bn(W, g, be, m, v):
    inv = (g.astype(np.float64) / np.sqrt(v.astype(np.float64) + EPS))
    Wp = (W.astype(np.float64) * inv[:, None, None, None]).astype(np.float32)
    bp = (be.astype(np.float64) - m.astype(np.float64) * inv).astype(np.float32)
    return Wp, bp


def _pack_weights(Wk, gk, bk, mk, vk, Ws, gs, bs, ms, vs, Wf, gf, bf, mf, vf):
    Wkp, bkp = _fold_bn(Wk, gk, bk, mk, vk)
    Wsp, bsp = _fold_bn(Ws, gs, bs, ms, vs)
    Wfp, bfp = _fold_bn(Wf, gf, bf, mf, vf)

    def pack33(Wp):  # [H, C, 3, 3] -> [k, (hg, t, cg), m]
        w = Wp.reshape(2, 128, 2, 128, 3, 3)  # hg, m, cg, k, dy, dx
        w = w.transpose(3, 0, 4, 5, 2, 1)  # k, hg, dy, dx, cg, m
        return np.ascontiguousarray(w.reshape(128, 36, 128))

    wk_h = pack33(Wkp)
    ws_h = pack33(Wsp).astype(np.float16)
    w = Wfp[:, :, 0, 0].reshape(2, 128, 2, 128)  # og, m, hg, k
    wf_h = np.ascontiguousarray(
        w.transpose(3, 2, 0, 1).reshape(128, 4, 128)).astype(np.float16)

    bias_h = np.zeros((128, 6), np.float32)
    bias_h[:, 0] = bkp[0:128]
    bias_h[:, 1] = bkp[128:256]
    bias_h[:, 2] = bsp[0:128]
    bias_h[:, 3] = bsp[128:256]
    bias_h[:, 4] = bfp[0:128]
    bias_h[:, 5] = bfp[128:256]
    eye_h = np.eye(128, dtype=np.float16)
    return wk_h, ws_h, wf_h, bias_h, eye_h


_NC_CACHE = {}


def _get_nc(nb):
    if nb not in _NC_CACHE:
        _NC_CACHE[nb] = _build_nc(nb)
    return _NC_CACHE[nb]


def run(inputs, trace=False):
    """Build in_maps, run on 8 cores, return (full_output, BassKernelResults)."""
    kernel = np.asarray(inputs["kernel"], np.float32)
    search = np.asarray(inputs["search"], np.float32)
    wk_h, ws_h, wf_h, bias_h, eye_h = _pack_weights(
        np.asarray(inputs["Wk"]), np.asarray(inputs["gk"]), np.asarray(inputs["bk"]),
        np.asarray(inputs["mk"]), np.asarray(inputs["vk"]),
        np.asarray(inputs["Ws"]), np.asarray(inputs["gs"]), np.asarray(inputs["bs"]),
        np.asarray(inputs["ms"]), np.asarray(inputs["vs"]),
        np.asarray(inputs["Wf"]), np.asarray(inputs["gf"]), np.asarray(inputs["bf"]),
        np.asarray(inputs["mf"]), np.asarray(inputs["vf"]),
    )
    nc = _get_nc(NB)
    # fp16 on host: identical to the on-device cast the kernel used to do
    search_p = np.zeros((B, C, 31, 32), np.float16)
    search_p[:, :, :, :31] = search
    in_maps = []
    for i in range(N_CORES):
        kk = kernel[i * NB:(i + 1) * NB].reshape(NB, 2, 128, 7, 7)
        kin_h = np.ascontiguousarray(kk.transpose(2, 1, 3, 4, 0))
        in_maps.append({
            "search": np.ascontiguousarray(search_p[i * NB:(i + 1) * NB]),
            "kin": kin_h,
            "wk": wk_h, "ws": ws_h, "wf": wf_h, "bias": bias_h, "eye": eye_h,
        })
    res = run_bass_kernel_spmd(
        nc, in_maps, core_ids=list(range(N_CORES)), trace=trace
    )
    out = np.concatenate([res.results[i]["out"] for i in range(N_CORES)], axis=0)
    return out, res


def kernel(**inputs):
    out, _ = run(inputs, trace=False)
    return out


# revision 20
# speedup vs baseline: 1.6033x; 1.0793x over previous
# Trainium2 Bass kernel for nn_DepthCorr (SiamRPN-style depthwise correlation head).
#
# Pipeline (per batch):
#   kf   = relu(bn(conv3x3(kernel, Wk)))   [C=256, 7,7]  -> [H=256, 5,5]
#   sf   = relu(bn(conv3x3(search, Ws)))   [C=256,31,31] -> [H=256,29,29]
#   corr = relu(dwxcorr(sf, kf))                         -> [H=256,25,25]
#   out  = relu(bn(conv1x1(corr, Wf)))                   -> [C=256,25,25]
#
# Sharding: pure data-parallel over batch (128 batches / 8 cores = 16 per core).
# BN is folded into conv weights + per-channel bias on the host.
#
# Engine split (the key idea vs the diag-matmul baseline): the dense convs
# (conv1/conv2/conv3) run on the tensor engine, while the depthwise xcorr --
# which can only use 128 of the PE's 16K MACs per cycle via diagonal weights --
# runs entirely on the Vector (DVE) and GpSimd engines as per-tap fused
# multiply-accumulates: scalar_tensor_tensor(acc, sf_window, kf[c,t], acc).
# The 25 taps are split between DVE (fast, 2x fp16 mode) and GpSimd; each
# accumulates its taps into its own fp16 accumulator, then DVE merges and
# the scalar engine applies relu.  This overlaps the whole xcorr with the
# next batch's conv2 on the PE, roughly halving the critical path.
#
# DVE 2x mode needs 4B-aligned step-1 operands, so odd-tj tap windows read
# from a one-column-shifted copy of sf (sfs) made by the scalar engine.

import numpy as np
from contextlib import ExitStack

import concourse.bass as bass
import concourse.mybir as mybir
import concourse.tile as tile
from concourse import bacc
from concourse.bass_utils import run_bass_kernel_spmd

B, C, H = 128, 256, 256
N_CORES = 8
NB = B // N_CORES  # batches per core
EPS = 1e-5
FP = mybir.dt.float32
FR = mybir.dt.float32r
RELU = mybir.ActivationFunctionType.Relu
COPY = mybir.ActivationFunctionType.Copy
F16 = mybir.dt.float16
MUL = mybir.AluOpType.mult
ADD = mybir.AluOpType.add

# xcorr tap split across engines (per-tap costs measured on HW, both hg):
#   PE    diag-matmul                    ~0.53us -> 8 taps (t=5..12 contiguous)
#   DVE   TS-mult (4x) + TT-add (2x)     ~1.60us -> 8 even-tj taps
#   ACT   mult (scale=kf) + DVE TT-add   ~1.79+0.84us -> 7 taps
#   Pool  TT mult+add (bcast kf)         ~7.1us  -> 2 taps
# The offloaded partials (acc fed by DVE adds, acc2 by Pool) are folded back
# into the PE's xcorr PSUM group with identity matmuls, so the existing relu
# activation epilogue merges everything for free.  The diag build (one big
# broadcast multiply) runs on the otherwise-idle Pool engine.
PE_TAPS = [(1, 0), (1, 1), (1, 2), (1, 3), (1, 4), (2, 0), (2, 1), (2, 2)]
PE_T0 = 5  # first PE tap index (contiguous run t=5..12 in kf's tap dim)
POOL_TAPS = [(3, 1), (3, 3)]
# TS taps restricted to tj in {0,4}: with 64B sf rows these windows stay
# 8B-aligned, which the DVE 4x tensor_scalar mode needs.
TS_TAPS = [(0, 0), (0, 4), (3, 0), (3, 4), (4, 0), (4, 4)]
AV_TAPS = [(0, 1), (0, 2), (0, 3), (2, 3), (2, 4), (3, 2), (4, 1), (4, 2), (4, 3)]


def _build_nc(nb=NB):
    nc = bacc.Bacc()

    search = nc.declare_dram_parameter("search", [nb, C, 31, 32], F16, isOutput=False)
    # kin pre-transposed on the host to [k, cg, h, w, b] so the DMA is contiguous
    kin = nc.declare_dram_parameter("kin", [128, 2, 7, 7, nb], FR, isOutput=False)
    wk_d = nc.declare_dram_parameter("wk", [128, 36, 128], FR, isOutput=False)
    ws_d = nc.declare_dram_parameter("ws", [128, 36, 128], F16, isOutput=False)
    wf_d = nc.declare_dram_parameter("wf", [128, 4, 128], F16, isOutput=False)
    bias_d = nc.declare_dram_parameter("bias", [128, 6], FP, isOutput=False)
    eye_d = nc.declare_dram_parameter("eye", [128, 128], F16, isOutput=False)
    out_d = nc.declare_dram_parameter("out", [nb, C, 25, 25], FP, isOutput=True)

    C2_SPLITS = [(0, 16), (16, 13)]  # conv2 row splits; N = 464 / 377
    O_SPLITS = [(0, 13), (13, 12)]  # conv3 row splits over corr; N = 325 / 300

    with tile.TileContext(nc) as tc, ExitStack() as ctx:
        wpool = ctx.enter_context(tc.tile_pool(name="wpool", bufs=1))
        kpool = ctx.enter_context(tc.tile_pool(name="kpool", bufs=1))
        spool = ctx.enter_context(tc.tile_pool(name="spool", bufs=3))
        fpool = ctx.enter_context(tc.tile_pool(name="fpool", bufs=3))
        dpool = ctx.enter_context(tc.tile_pool(name="dpool", bufs=3))
        apool = ctx.enter_context(tc.tile_pool(name="apool", bufs=3))
        bpool = ctx.enter_context(tc.tile_pool(name="bpool", bufs=3))
        cpool = ctx.enter_context(tc.tile_pool(name="cpool", bufs=2))
        tpool = ctx.enter_context(tc.tile_pool(name="tpool", bufs=8))
        p2pool = ctx.enter_context(tc.tile_pool(name="p2pool", bufs=2))
        opool = ctx.enter_context(tc.tile_pool(name="opool", bufs=2))
        ps_c = ctx.enter_context(tc.tile_pool(name="ps_c", bufs=3, space="PSUM"))
        ps_x = ctx.enter_context(tc.tile_pool(name="ps_x", bufs=3, space="PSUM"))
        ps_o = ctx.enter_context(tc.tile_pool(name="ps_o", bufs=2, space="PSUM"))

        # --- search prefetch (sync queue; weights go on gpsimd queue) ---
        s_tiles = {}

        def load_search(b):
            s_sb = spool.tile([128, 2, 31, 32], F16, tag="sin")
            nc.sync.dma_start(out=s_sb[:, 0, :, :], in_=search[b, 0:128, :, :])
            nc.scalar.dma_start(out=s_sb[:, 1, :, :], in_=search[b, 128:256, :, :])
            s_tiles[b] = s_sb

        # --- resident constants; conv2 weights + bias race ahead of the
        # search prefetch so the PE can start within ~10us ---
        wk_sb = wpool.tile([128, 36, 128], FR, tag="wk")
        ws_sb = wpool.tile([128, 36, 128], F16, tag="ws")
        wf_sb = wpool.tile([128, 4, 128], F16, tag="wf")
        bias_sb = wpool.tile([128, 6], FP, tag="bias")
        eye_sb = wpool.tile([128, 128], F16, tag="eye")
        load_search(0)
        nc.gpsimd.dma_start(out=ws_sb[:], in_=ws_d[:])
        nc.scalar.dma_start(out=bias_sb[:], in_=bias_d[:])
        nc.scalar.dma_start(out=eye_sb[:], in_=eye_d[:])
        k_sbs = []
        for cg in range(2):
            k_sb = kpool.tile([128, 7, 7, nb], FR, tag=f"kin{cg}")
            k_sbs.append(k_sb)
        # kf_sb[h_part, hg, tap, b]
        kf_sb = kpool.tile([128, 2, 25, nb], FP, tag="kf")

        def load_deferred_consts():
            nc.gpsimd.dma_start(out=wk_sb[:], in_=wk_d[:])
            nc.gpsimd.dma_start(out=wf_sb[:], in_=wf_d[:])
            for cg in range(2):
                nc.gpsimd.dma_start(out=k_sbs[cg][:], in_=kin[:, cg])

        def conv1():
            for hg in range(2):
                ps = ps_c.tile([128, 5, 5, nb], FP, tag="psc")
                n_mm = 0
                for cg in range(2):
                    for dy in range(3):
                        for dx in range(3):
                            t = dy * 3 + dx
                            nc.tensor.matmul(
                                ps[:],
                                lhsT=wk_sb[:, hg * 18 + t * 2 + cg, :],
                                rhs=k_sbs[cg][:, dy:dy + 5, dx:dx + 5, :],
                                start=(n_mm == 0),
                                stop=(n_mm == 17),
                            )
                            n_mm += 1
                nc.scalar.activation(
                    out=kf_sb[:, hg, :, :],
                    in_=ps.rearrange("p a b c -> p (a b) c"),
                    func=RELU,
                    bias=bias_sb[:, 0 + hg:1 + hg],
                    scale=1.0,
                )

        def conv3_and_store(b, corr):
            out_sb = opool.tile([128, 2, 25, 25], FP, tag="osb")
            for og in range(2):
                for (r0, nr) in O_SPLITS:
                    ps = ps_o.tile([128, nr, 25], FP, tag="pso")
                    for hg in range(2):
                        nc.tensor.matmul(
                            ps[:],
                            lhsT=wf_sb[:, hg * 2 + og, :],
                            rhs=corr[:, hg, r0:r0 + nr, 0:25],
                            start=(hg == 0),
                            stop=(hg == 1),
                        )
                    nc.scalar.activation(
                        out=out_sb[:, og, r0:r0 + nr, :],
                        in_=ps[:],
                        func=RELU,
                        bias=bias_sb[:, 4 + og:5 + og],
                        scale=1.0,
                    )
                nc.sync.dma_start(
                    out=out_d[b, og * 128:(og + 1) * 128, :, :],
                    in_=out_sb[:, og, :, :],
                )

        def xcorr_pe(b, sf, diag, acc, acc2):
            # PE taps accumulate in PSUM, then the offloaded partials are
            # added with identity matmuls; relu epilogue merges to corr.
            corr = cpool.tile([128, 2, 25, 32], F16, tag="corr")
            for hg in range(2):
                for (r0, nr) in O_SPLITS:
                    ps = ps_x.tile([128, nr, 25], FP, tag="psx")
                    for i, (ti, tj) in enumerate(PE_TAPS):
                        nc.tensor.matmul(
                            ps[:],
                            lhsT=diag[:, hg, i, :],
                            rhs=sf[:, hg, ti + r0:ti + r0 + nr, tj:tj + 25],
                            start=(i == 0),
                            stop=False,
                        )
                    nc.tensor.matmul(
                        ps[:], lhsT=eye_sb[:],
                        rhs=acc[:, hg, r0:r0 + nr, 0:25],
                        start=False, stop=False,
                    )
                    nc.tensor.matmul(
                        ps[:], lhsT=eye_sb[:],
                        rhs=acc2[:, hg, r0:r0 + nr, 0:25],
                        start=False, stop=True,
                    )
                    nc.scalar.activation(
                        out=corr[:, hg, r0:r0 + nr, 0:25],
                        in_=ps[:],
                        func=RELU,
                        scale=1.0,
                    )
            return corr

        # --- per-batch main pipeline, software-pipelined with lag 2: the PE
        # xcorr + conv3 for batch b-2 are emitted after batch b's conv2 and
        # offloaded taps, so the PE never waits on the slower xcorr engines. ---
        state = {}
        for b in range(nb):
            if b + 1 < nb:
                load_search(b + 1)
            s_sb = s_tiles.pop(b)

            if b == 0:
                load_deferred_consts()

            # conv2: search branch -> sf [h_part, hg, 29, 30] (col 29 unused pad)
            sf = fpool.tile([128, 2, 29, 32], F16, tag="sf")
            for hg in range(2):
                for (y0, ny) in C2_SPLITS:
                    ps = ps_c.tile([128, ny, 29], FP, tag="psc")
                    n_mm = 0
                    for cg in range(2):
                        for dy in range(3):
                            for dx in range(3):
                                t = dy * 3 + dx
                                nc.tensor.matmul(
                                    ps[:],
                                    lhsT=ws_sb[:, hg * 18 + t * 2 + cg, :],
                                    rhs=s_sb[
                                        :, cg, dy + y0:dy + y0 + ny, dx:dx + 29
                                    ],
                                    start=(n_mm == 0),
                                    stop=(n_mm == 17),
                                )
                                n_mm += 1
                    nc.scalar.activation(
                        out=sf[:, hg, y0:y0 + ny, 0:29],
                        in_=ps[:],
                        func=RELU,
                        bias=bias_sb[:, 2 + hg:3 + hg],
                        scale=1.0,
                    )
            if b == 0:
                conv1()

            # xcorr-PE + conv3 for batch b-2 go ahead of this batch's offload
            # mults in every engine queue, so the PE's dependencies (the relu
            # epilogues on the scalar engine) are never stuck behind them.
            if b >= 2:
                corr = xcorr_pe(b - 2, *state.pop(b - 2))
                conv3_and_store(b - 2, corr)

            # diag for the PE taps: diag[c, hg, i, m] = kf[c, t5+i] * (c == m)
            # (built on the Pool engine; 1x broadcast multiply)
            diag = dpool.tile([128, 2, len(PE_TAPS), 128], F16, tag="diag")
            nc.gpsimd.tensor_mul(
                diag[:],
                kf_sb[:, :, PE_T0:PE_T0 + len(PE_TAPS), b]
                .unsqueeze(3).broadcast_to([128, 2, len(PE_TAPS), 128]),
                eye_sb.unsqueeze(1).unsqueeze(1)
                .broadcast_to([128, 2, len(PE_TAPS), 128]),
            )

            # offloaded xcorr taps -> acc (DVE adds) and acc2 (Pool)
            acc = apool.tile([128, 2, 25, 32], F16, tag="acc")
            acc2 = bpool.tile([128, 2, 25, 32], F16, tag="acc2")
            for i, (ti, tj) in enumerate(AV_TAPS):
                t = ti * 5 + tj
                dst = acc if i == 0 else tpool.tile([128, 2, 25, 32], F16, tag="tmp")
                for hg in range(2):
                    nc.scalar.activation(
                        out=dst[:, hg, :, 0:25],
                        in_=sf[:, hg, ti:ti + 25, tj:tj + 25],
                        func=COPY,
                        scale=kf_sb[:, hg, t, b:b + 1],
                    )
                if i > 0:
                    nc.vector.tensor_add(
                        acc[:, :, :, 0:25], acc[:, :, :, 0:25], dst[:, :, :, 0:25]
                    )
            for (ti, tj) in TS_TAPS:
                t = ti * 5 + tj
                tmp = tpool.tile([128, 2, 25, 32], F16, tag="tmp")
                for hg in range(2):
                    nc.vector.tensor_scalar_mul(
                        tmp[:, hg, :, 0:25],
                        sf[:, hg, ti:ti + 25, tj:tj + 25],
                        kf_sb[:, hg, t, b:b + 1],
                    )
                nc.vector.tensor_add(
                    acc[:, :, :, 0:25], acc[:, :, :, 0:25], tmp[:, :, :, 0:25]
                )
            for i, (ti, tj) in enumerate(POOL_TAPS):
                t = ti * 5 + tj
                for hg in range(2):
                    kb = (
                        kf_sb[:, hg, t, b:b + 1]
                        .unsqueeze(1).broadcast_to([128, 25, 25])
                    )
                    in0 = sf[:, hg, ti:ti + 25, tj:tj + 25]
                    if i == 0:
                        nc.gpsimd.tensor_mul(acc2[:, hg, :, 0:25], in0, kb)
                    else:
                        tmp2 = p2pool.tile([128, 25, 25], F16, tag="tmp2")
                        nc.gpsimd.tensor_mul(tmp2[:], in0, kb)
                        nc.gpsimd.tensor_add(
                            acc2[:, hg, :, 0:25], acc2[:, hg, :, 0:25], tmp2[:]
                        )
            state[b] = (sf, diag, acc, acc2)
        for bb in (nb - 2, nb - 1):
            corr = xcorr_pe(bb, *state.pop(bb))
            conv3_and_store(bb, corr)

    nc.compile()
    return nc


def _fold_bn(W, g, be, m, v):
    inv = (g.astype(np.float64) / np.sqrt(v.astype(np.float64) + EPS))
    Wp = (W.astype(np.float64) * inv[:, None, None, None]).astype(np.float32)
    bp = (be.astype(np.float64) - m.astype(np.float64) * inv).astype(np.float32)
    return Wp, bp


def _pack_weights(Wk, gk, bk, mk, vk, Ws, gs, bs, ms, vs, Wf, gf, bf, mf, vf):
    Wkp, bkp = _fold_bn(Wk, gk, bk, mk, vk)
    Wsp, bsp = _fold_bn(Ws, gs, bs, ms, vs)
    Wfp, bfp = _fold_bn(Wf, gf, bf, mf, vf)

    def pack33(Wp):  # [H, C, 3, 3] -> [k, (hg, t, cg), m]
        w = Wp.reshape(2, 128, 2, 128, 3, 3)  # hg, m, cg, k, dy, dx
        w = w.transpose(3, 0, 4, 5, 2, 1)  # k, hg, dy, dx, cg, m
        return np.ascontiguousarray(w.reshape(128, 36, 128))

    wk_h = pack33(Wkp)
    ws_h = pack33(Wsp).astype(np.float16)
    w = Wfp[:, :, 0, 0].reshape(2, 128, 2, 128)  # og, m, hg, k
    wf_h = np.ascontiguousarray(
        w.transpose(3, 2, 0, 1).reshape(128, 4, 128)).astype(np.float16)

    bias_h = np.zeros((128, 6), np.float32)
    bias_h[:, 0] = bkp[0:128]
    bias_h[:, 1] = bkp[128:256]
    bias_h[:, 2] = bsp[0:128]
    bias_h[:, 3] = bsp[128:256]
    bias_h[:, 4] = bfp[0:128]
    bias_h[:, 5] = bfp[128:256]
    eye_h = np.eye(128, dtype=np.float16)
    return wk_h, ws_h, wf_h, bias_h, eye_h


_NC_CACHE = {}


def _get_nc(nb):
    if nb not in _NC_CACHE:
        _NC_CACHE[nb] = _build_nc(nb)
    return _NC_CACHE[nb]


def run(inputs, trace=False):
    """Build in_maps, run on 8 cores, return (full_output, BassKernelResults)."""
    kernel = np.asarray(inputs["kernel"], np.float32)
    search = np.asarray(inputs["search"], np.float32)
    wk_h, ws_h, wf_h, bias_h, eye_h = _pack_weights(
        np.asarray(inputs["Wk"]), np.asarray(inputs["gk"]), np.asarray(inputs["bk"]),
        np.asarray(inputs["mk"]), np.asarray(inputs["vk"]),
        np.asarray(inputs["Ws"]), np.asarray(inputs["gs"]), np.asarray(inputs["bs"]),
        np.asarray(inputs["ms"]), np.asarray(inputs["vs"]),
        np.asarray(inputs["Wf"]), np.asarray(inputs["gf"]), np.asarray(inputs["bf"]),
        np.asarray(inputs["mf"]), np.asarray(inputs["vf"]),
    )
    nc = _get_nc(NB)
    # fp16 on host: identical to the on-device cast the kernel used to do
    search_p = np.zeros((B, C, 31, 32), np.float16)
    search_p[:, :, :, :31] = search
    in_maps = []
    for i in range(N_CORES):
        kk = kernel[i * NB:(i + 1) * NB].reshape(NB, 2, 128, 7, 7)
        kin_h = np.ascontiguousarray(kk.transpose(2, 1, 3, 4, 0))
        in_maps.append({
            "search": np.ascontiguousarray(search_p[i * NB:(i + 1) * NB]),
            "kin": kin_h,
            "wk": wk_h, "ws": ws_h, "wf": wf_h, "bias": bias_h, "eye": eye_h,
        })
    res = run_bass_kernel_spmd(
        nc, in_maps, core_ids=list(range(N_CORES)), trace=trace
    )
    out = np.concatenate([res.results[i]["out"] for i in range(N_CORES)], axis=0)
    return out, res


def kernel(**inputs):
    out, _ = run(inputs, trace=False)
    return out
